# revision 35
# baseline (speedup 1.0000x reference)
"""Arcface pairwise-similarity loss kernel for one TRN2 chip (8 NeuronCores).

Reference computation (per batch element b):
  xn = x / ||x||_2 (over H)                      x: [S=5, W=1024, H=128]
  for each pair p=(i,j) of the 5 S-slices:
    scores[p] = 5 * xn[i] @ xn[j].T              -> [W, W]
    ret[p][w] = first v with target[j][v] == target[i][w], else 0

Sharding: data-parallel over B=8, one batch element per NeuronCore. No
cross-core communication; outputs are concatenated on the host.

Per-core device pipeline:
  Stage A: one DMA per s loads x[s] as [128, 8, 128] (partition = w%128);
           sum of squares / sqrt / reciprocal / scale are batched over
           the whole s (broadcast access patterns), then 8 PE transposes
           build xt[s] = [128(H), 1024(W)] bf16 matmul operands.
  Stage B: 10 pairs x 8 m-tiles x 2 bf16 matmuls [K=128, M=128, N=512].
           Full [128, 1024] PSUM tiles are evacuated with the x5 scale
           folded in, alternating ScalarE / VectorE; 4 m-tiles are
           staged per SBUF buffer so each output DMA writes a
           contiguous 2 MiB block.
  Stage C: ret_target without the O(W^2) argmax: per s a one-hot class
           map OH[c, v] = (target[s][v] == c) (64 classes, fp16). The
           first matching index per class depends only on j: 4 masked
           min-reductions. The per-w gather is an fp16 matmul grouped
           by i (lhsT = ADJS[:, i:4]), exact because integers <= 2048
           are exact in fp16.
"""

import os
import sys

if "/opt/trn_rl_repo" not in sys.path:
    sys.path.insert(0, "/opt/trn_rl_repo")

import numpy as np

B, S, W, H = 8, 5, 1024, 128
PAIRS = [(0, 1), (0, 2), (0, 3), (0, 4), (1, 2), (1, 3), (1, 4), (2, 3), (2, 4), (3, 4)]
P = len(PAIRS)
N_CORES = 8
NCLASS = 64
BIG = 2048.0  # > any valid index; exact in fp16/fp32

MM_DT = os.environ.get("K_MM_DT", "bf16")  # "bf16" | "f32r"

_CACHE = {}


def _build():
    import concourse.bass as bass
    import concourse.tile as tile
    from concourse import bacc, mybir
    from concourse.masks import make_identity

    f32 = mybir.dt.float32
    f16 = mybir.dt.float16
    mm_dt = mybir.dt.bfloat16 if MM_DT == "bf16" else mybir.dt.float32r
    i32 = mybir.dt.int32
    Alu = mybir.AluOpType
    Act = mybir.ActivationFunctionType

    nc = bacc.Bacc("TRN2", target_bir_lowering=False, debug=False, num_devices=N_CORES)

    x_h = nc.dram_tensor("x", [S, W, H], f32, kind="ExternalInput")
    t_h = nc.dram_tensor("tgt", [S, W], f32, kind="ExternalInput")
    out_dt = f32 if os.environ.get("K_OUT_DT", "bf16") == "f32" else mybir.dt.bfloat16
    sc_h = nc.dram_tensor("scores", [P, W, W], out_dt, kind="ExternalOutput")
    rt_h = nc.dram_tensor("ret", [P, W], i32, kind="ExternalOutput")

    MT = W // 128  # 8 m-tiles per pair
    MG = 4  # m-tiles staged per output DMA

    with tile.TileContext(nc) as tc:
        with (
            tc.tile_pool(name="persist", bufs=1) as persist,
            tc.tile_pool(name="xin", bufs=2) as xin_pool,
            tc.tile_pool(name="scr", bufs=3) as scr_pool,
            tc.tile_pool(name="small", bufs=4) as small_pool,
            tc.tile_pool(name="outp", bufs=10) as out_pool,
            tc.tile_pool(name="ps", bufs=4, space="PSUM") as ps_pool,
            tc.tile_pool(name="tb", bufs=2) as tb_pool,
        ):
            ident = persist.tile([128, 128], mm_dt, tag="ident")
            make_identity(nc, ident)

            # viota[c, v] = v - BIG (constant across partitions)
            viota = persist.tile([NCLASS, W], f16, tag="viota")
            nc.gpsimd.iota(
                viota,
                pattern=[[1, W]],
                base=int(-BIG),
                channel_multiplier=0,
                allow_small_or_imprecise_dtypes=True,
            )
            # ciota[c, 0] = c
            ciota = persist.tile([NCLASS, 1], f32, tag="ciota")
            nc.gpsimd.iota(
                ciota,
                pattern=[[1, 1]],
                base=0,
                channel_multiplier=1,
                allow_small_or_imprecise_dtypes=True,
            )

            xt = [
                persist.tile([128, W], mm_dt, tag=f"xt{s}", name=f"xt{s}")
                for s in range(S)
            ]
            oh = [
                persist.tile([NCLASS, W], f16, tag=f"oh{s}", name=f"oh{s}")
                for s in range(S)
            ]

            # ---- Stage A + B interleaved: as soon as both slices of a
            # pair are normalized+transposed, emit its matmuls so output
            # DMAs start early. ----
            def stage_a(s):
                xin = xin_pool.tile([128, MT, H], f32, tag="xin")
                # partition = w % 128, free = (m, h)
                nc.sync.dma_start(
                    out=xin, in_=x_h[s].rearrange("(m q) h -> q m h", q=128)
                )
                scr = scr_pool.tile([128, MT, H], f32, tag="scr")
                nc.gpsimd.tensor_mul(scr, xin, xin)
                ss = small_pool.tile([128, MT], f32, tag="ss")
                nc.vector.tensor_reduce(ss, scr, axis=mybir.AxisListType.X, op=Alu.add)
                sq = small_pool.tile([128, MT], f32, tag="sq")
                nc.scalar.activation(out=sq, in_=ss, func=Act.Sqrt)
                rn = small_pool.tile([128, MT], f32, tag="rn")
                nc.vector.reciprocal(out=rn, in_=sq)
                # xn = x * rn, rn broadcast along h
                rnb = bass.AP(
                    tensor=rn.tensor,
                    offset=rn.offset,
                    ap=[*rn.ap, [0, H]],
                )
                xn = scr_pool.tile([128, MT, H], mm_dt, tag="xn")
                nc.gpsimd.tensor_mul(xn, xin, rnb)
                for m in range(MT):
                    nc.sync.dma_start_transpose(
                        out=xt[s][:, m * 128 : (m + 1) * 128], in_=xn[:, m, :]
                    )
                # one-hot class map for this s (fp16)
                tb = tb_pool.tile([NCLASS, W], f32, tag="tb")
                nc.gpsimd.dma_start(
                    out=tb, in_=t_h[s : s + 1, :].to_broadcast([NCLASS, W])
                )
                nc.gpsimd.tensor_scalar(
                    out=oh[s], in0=tb, scalar1=ciota, scalar2=None, op0=Alu.is_equal
                )

            ndma = 0

            def mm_tile(p, i, j, m):
                # one [128, 1024] output tile of pair p at row block m
                nonlocal ndma
                msl = slice(m * 128, (m + 1) * 128)
                ps = ps_pool.tile([128, W], f32, tag="ps")
                lhsT = xt[i][:, msl]
                nc.tensor.matmul(
                    ps[:, 0:512], lhsT, xt[j][:, 0:512], start=True, stop=True
                )
                nc.tensor.matmul(
                    ps[:, 512:1024], lhsT, xt[j][:, 512:1024], start=True, stop=True
                )
                so = out_pool.tile([128, W], out_dt, tag="so")
                if ndma % 16 < 9:
                    nc.scalar.activation(out=so, in_=ps, func=Act.Copy, scale=5.0)
                else:
                    nc.vector.tensor_scalar_mul(so, ps, 5.0)
                eng = (nc.scalar, nc.gpsimd)[ndma % 2]
                ndma += 1
                eng.dma_start(out=sc_h[p, msl, :], in_=so)

            pair_idx = {pr: n for n, pr in enumerate(PAIRS)}
            for s in range(S):
                stage_a(s)
                for i in range(s):
                    for m in range(MT):
                        mm_tile(pair_idx[(i, s)], i, s, m)

            # ---- Stage C part 2: first-match indices (per s), gather by i ----
            FIM = persist.tile([NCLASS, 4], f16, tag="FIM")
            for s in range(1, 5):
                scv = scr_pool.tile([NCLASS, W], f16, tag="scv")
                # scv[c,v] = OH[s][c,v] * (v - BIG); min_v -> first idx - BIG
                nc.vector.tensor_mul(scv, oh[s], viota)
                nc.vector.tensor_reduce(
                    FIM[:, s - 1 : s], scv, axis=mybir.AxisListType.X, op=Alu.min
                )
            FI = small_pool.tile([NCLASS, 4], f16, tag="FI")
            nc.vector.tensor_scalar_add(FI, FIM, BIG)  # first idx, or BIG
            IND = small_pool.tile([NCLASS, 4], f16, tag="IND")
            nc.vector.tensor_scalar(
                out=IND, in0=FI, scalar1=BIG, scalar2=None, op0=Alu.is_lt
            )
            ADJS = small_pool.tile([NCLASS, 4], f16, tag="ADJS")
            nc.vector.tensor_mul(ADJS, FI, IND)  # BIG -> 0 (argmax default)

            row_off = [0, 4, 7, 9]
            for i in range(4):
                rows = 4 - i
                psg = ps_pool.tile([rows, W], f32, tag="ps")
                # out rows r -> pair (i, i+1+r); PAIRS order preserved
                nc.tensor.matmul(
                    psg[:, 0:512], ADJS[:, i:4], oh[i][:, 0:512],
                    start=True, stop=True,
                )
                nc.tensor.matmul(
                    psg[:, 512:1024], ADJS[:, i:4], oh[i][:, 512:1024],
                    start=True, stop=True,
                )
                ri = small_pool.tile([rows, W], i32, tag=f"ri{i}", name=f"ri{i}")
                nc.vector.tensor_copy(out=ri, in_=psg)
                nc.sync.dma_start(
                    out=rt_h[row_off[i] : row_off[i] + rows, :], in_=ri
                )

    nc.compile()
    return nc


def _get_nc():
    if "nc" not in _CACHE:
        _CACHE["nc"] = _build()
    return _CACHE["nc"]


def _run(inputs, trace=False, **kw):
    from concourse.bass_utils import run_bass_kernel_spmd

    nc = _get_nc()
    x = np.ascontiguousarray(np.asarray(inputs["x"], dtype=np.float32))
    t = np.ascontiguousarray(np.asarray(inputs["target"]).astype(np.float32))
    assert x.shape == (B, S, W, H) and t.shape == (B, S, W)
    in_maps = [{"x": x[b], "tgt": t[b]} for b in range(N_CORES)]
    r = run_bass_kernel_spmd(nc, in_maps, list(range(N_CORES)), trace=trace, **kw)
    scores = np.concatenate(
        [
            np.asarray(r.results[b]["scores"], dtype=np.float32).reshape(P * W, W)
            for b in range(N_CORES)
        ],
        axis=0,
    )
    ret = np.concatenate(
        [r.results[b]["ret"].reshape(-1) for b in range(N_CORES)], axis=0
    ).astype(np.int32)
    return scores, ret, r


def kernel(**inputs):
    scores, ret, _ = _run(inputs, trace=False)
    return scores, ret


if __name__ == "__main__":
    nc = _get_nc()
    print("built + compiled OK")


# revision 39
# speedup vs baseline: 2.1024x; 2.1024x over previous
"""Arcface pairwise-similarity loss kernel for one TRN2 chip (8 NeuronCores).

Reference computation (per batch element b):
  xn = x / ||x||_2 (over H)                      x: [S=5, W=1024, H=128]
  for each pair p=(i,j) of the 5 S-slices:
    scores[p] = 5 * xn[i] @ xn[j].T              -> [W, W]
    ret[p][w] = first v with target[j][v] == target[i][w], else 0

Sharding: data-parallel over B=8, one batch element per NeuronCore. No
cross-core communication; outputs are concatenated on the host.

Per-core device pipeline:
  Stage A: one DMA per s loads x[s] as [128, 8, 128] (partition = w%128);
           sum of squares / sqrt / reciprocal / scale are batched over
           the whole s (broadcast access patterns), then 8 PE transposes
           build xt[s] = [128(H), 1024(W)] bf16 matmul operands.
  Stage B: 10 pairs x 8 m-tiles x 2 bf16 matmuls [K=128, M=128, N=512].
           Full [128, 1024] PSUM tiles are evacuated with the x5 scale
           folded in, alternating ScalarE / VectorE; 4 m-tiles are
           staged per SBUF buffer so each output DMA writes a
           contiguous 2 MiB block.
  Stage C: ret_target without the O(W^2) argmax: per s a one-hot class
           map OH[c, v] = (target[s][v] == c) (64 classes, fp16). The
           first matching index per class depends only on j: 4 masked
           min-reductions. The per-w gather is an fp16 matmul grouped
           by i (lhsT = ADJS[:, i:4]), exact because integers <= 2048
           are exact in fp16.
"""

import os
import sys

if "/opt/trn_rl_repo" not in sys.path:
    sys.path.insert(0, "/opt/trn_rl_repo")

import numpy as np

B, S, W, H = 8, 5, 1024, 128
PAIRS = [(0, 1), (0, 2), (0, 3), (0, 4), (1, 2), (1, 3), (1, 4), (2, 3), (2, 4), (3, 4)]
P = len(PAIRS)
N_CORES = 8
NCLASS = 64
BIG = 2048.0  # > any valid index; exact in fp16/fp32

MM_DT = os.environ.get("K_MM_DT", "bf16")  # "bf16" | "f32r"

_CACHE = {}


def _build():
    import concourse.bass as bass
    import concourse.tile as tile
    from concourse import bacc, mybir
    from concourse.masks import make_identity

    f32 = mybir.dt.float32
    f16 = mybir.dt.float16
    mm_dt = mybir.dt.bfloat16 if MM_DT == "bf16" else mybir.dt.float32r
    i32 = mybir.dt.int32
    Alu = mybir.AluOpType
    Act = mybir.ActivationFunctionType

    nc = bacc.Bacc("TRN2", target_bir_lowering=False, debug=False, num_devices=N_CORES)

    x_h = nc.dram_tensor("x", [S, W, H], f32, kind="ExternalInput")
    t_h = nc.dram_tensor("tgt", [S, W], f32, kind="ExternalInput")
    out_dt = f32 if os.environ.get("K_OUT_DT", "bf16") == "f32" else mybir.dt.bfloat16
    sc_h = nc.dram_tensor("scores", [P, W, W], out_dt, kind="ExternalOutput")
    rt_h = nc.dram_tensor("ret", [P, W], i32, kind="ExternalOutput")

    MT = W // 128  # 8 m-tiles per pair
    MG = 4  # m-tiles staged per output DMA

    with tile.TileContext(nc) as tc:
        with (
            tc.tile_pool(name="persist", bufs=1) as persist,
            tc.tile_pool(name="xin", bufs=2) as xin_pool,
            tc.tile_pool(name="scr", bufs=3) as scr_pool,
            tc.tile_pool(name="small", bufs=4) as small_pool,
            tc.tile_pool(name="outp", bufs=10) as out_pool,
            tc.tile_pool(name="ps", bufs=8, space="PSUM") as ps_pool,
            tc.tile_pool(name="tb", bufs=2) as tb_pool,
        ):
            ident = persist.tile([128, 128], mm_dt, tag="ident")
            make_identity(nc, ident)

            # viota[c, v] = v - BIG (constant across partitions)
            viota = persist.tile([NCLASS, W], f16, tag="viota")
            nc.gpsimd.iota(
                viota,
                pattern=[[1, W]],
                base=int(-BIG),
                channel_multiplier=0,
                allow_small_or_imprecise_dtypes=True,
            )
            # ciota[c, 0] = c
            ciota = persist.tile([NCLASS, 1], f32, tag="ciota")
            nc.gpsimd.iota(
                ciota,
                pattern=[[1, 1]],
                base=0,
                channel_multiplier=1,
                allow_small_or_imprecise_dtypes=True,
            )

            xt = [
                persist.tile([128, W], mm_dt, tag=f"xt{s}", name=f"xt{s}")
                for s in range(S)
            ]
            oh = [
                persist.tile([NCLASS, W], f16, tag=f"oh{s}", name=f"oh{s}")
                for s in range(S)
            ]

            # ---- Stage A + B interleaved: as soon as both slices of a
            # pair are normalized+transposed, emit its matmuls so output
            # DMAs start early. ----
            def stage_a(s):
                xin = xin_pool.tile([128, MT, H], f32, tag="xin")
                # partition = w % 128, free = (m, h)
                nc.sync.dma_start(
                    out=xin, in_=x_h[s].rearrange("(m q) h -> q m h", q=128)
                )
                scr = scr_pool.tile([128, MT, H], f32, tag="scr")
                nc.vector.tensor_mul(scr, xin, xin)
                ss = small_pool.tile([128, MT], f32, tag="ss")
                nc.vector.tensor_reduce(ss, scr, axis=mybir.AxisListType.X, op=Alu.add)
                sq = small_pool.tile([128, MT], f32, tag="sq")
                nc.scalar.activation(out=sq, in_=ss, func=Act.Sqrt)
                rn = small_pool.tile([128, MT], f32, tag="rn")
                nc.vector.reciprocal(out=rn, in_=sq)
                # xn = x * rn, rn broadcast along h
                rnb = bass.AP(
                    tensor=rn.tensor,
                    offset=rn.offset,
                    ap=[*rn.ap, [0, H]],
                )
                xn = scr_pool.tile([128, MT, H], mm_dt, tag="xn")
                nc.vector.tensor_mul(xn, xin, rnb)
                for m in range(MT):
                    pt = ps_pool.tile([128, 128], mm_dt, tag="ps")
                    nc.tensor.transpose(pt, xn[:, m, :], ident)
                    nc.any.tensor_copy(out=xt[s][:, m * 128 : (m + 1) * 128], in_=pt)
                # one-hot class map for this s (fp16)
                tb = tb_pool.tile([NCLASS, W], f32, tag="tb")
                nc.gpsimd.dma_start(
                    out=tb, in_=t_h[s : s + 1, :].to_broadcast([NCLASS, W])
                )
                nc.vector.tensor_scalar(
                    out=oh[s], in0=tb, scalar1=ciota, scalar2=None, op0=Alu.is_equal
                )

            ndma = 0

            def mm_tile(p, i, j, m):
                # one [128, 1024] output tile of pair p at row block m;
                # two independent single-bank PSUM halves so the PE never
                # waits on a whole-tile evacuation
                nonlocal ndma
                msl = slice(m * 128, (m + 1) * 128)
                lhsT = xt[i][:, msl]
                so = out_pool.tile([128, W], out_dt, tag="so")
                for h in range(2):
                    hsl = slice(h * 512, (h + 1) * 512)
                    ph = ps_pool.tile([128, 512], f32, tag="ps")
                    nc.tensor.matmul(ph, lhsT, xt[j][:, hsl], start=True, stop=True)
                    if (2 * ndma + h) % 16 < 9:
                        nc.scalar.activation(
                            out=so[:, hsl], in_=ph, func=Act.Copy, scale=5.0
                        )
                    else:
                        nc.vector.tensor_scalar_mul(so[:, hsl], ph, 5.0)
                eng = (nc.sync, nc.scalar, nc.gpsimd)[ndma % 3]
                ndma += 1
                eng.dma_start(out=sc_h[p, msl, :], in_=so)

            pair_idx = {pr: n for n, pr in enumerate(PAIRS)}
            for s in range(S):
                stage_a(s)
                for i in range(s):
                    for m in range(MT):
                        mm_tile(pair_idx[(i, s)], i, s, m)

            # ---- Stage C part 2: first-match indices (per s), gather by i ----
            FIM = persist.tile([NCLASS, 4], f16, tag="FIM")
            for s in range(1, 5):
                scv = scr_pool.tile([NCLASS, W], f16, tag="scv")
                # scv[c,v] = OH[s][c,v] * (v - BIG); min_v -> first idx - BIG
                nc.vector.tensor_mul(scv, oh[s], viota)
                nc.vector.tensor_reduce(
                    FIM[:, s - 1 : s], scv, axis=mybir.AxisListType.X, op=Alu.min
                )
            FI = small_pool.tile([NCLASS, 4], f16, tag="FI")
            nc.vector.tensor_scalar_add(FI, FIM, BIG)  # first idx, or BIG
            IND = small_pool.tile([NCLASS, 4], f16, tag="IND")
            nc.vector.tensor_scalar(
                out=IND, in0=FI, scalar1=BIG, scalar2=None, op0=Alu.is_lt
            )
            ADJS = small_pool.tile([NCLASS, 4], f16, tag="ADJS")
            nc.vector.tensor_mul(ADJS, FI, IND)  # BIG -> 0 (argmax default)

            row_off = [0, 4, 7, 9]
            for i in range(4):
                rows = 4 - i
                ri = small_pool.tile([rows, W], i32, tag=f"ri{i}", name=f"ri{i}")
                # out rows r -> pair (i, i+1+r); PAIRS order preserved
                for h in range(2):
                    hsl = slice(h * 512, (h + 1) * 512)
                    psg = ps_pool.tile([rows, 512], f32, tag="ps")
                    nc.tensor.matmul(
                        psg, ADJS[:, i:4], oh[i][:, hsl], start=True, stop=True
                    )
                    nc.vector.tensor_copy(out=ri[:, hsl], in_=psg)
                nc.sync.dma_start(
                    out=rt_h[row_off[i] : row_off[i] + rows, :], in_=ri
                )

    nc.compile()
    return nc


def _get_nc():
    if "nc" not in _CACHE:
        _CACHE["nc"] = _build()
    return _CACHE["nc"]


def _run(inputs, trace=False, **kw):
    from concourse.bass_utils import run_bass_kernel_spmd

    nc = _get_nc()
    x = np.ascontiguousarray(np.asarray(inputs["x"], dtype=np.float32))
    t = np.ascontiguousarray(np.asarray(inputs["target"]).astype(np.float32))
    assert x.shape == (B, S, W, H) and t.shape == (B, S, W)
    in_maps = [{"x": x[b], "tgt": t[b]} for b in range(N_CORES)]
    r = run_bass_kernel_spmd(nc, in_maps, list(range(N_CORES)), trace=trace, **kw)
    scores = np.concatenate(
        [
            np.asarray(r.results[b]["scores"], dtype=np.float32).reshape(P * W, W)
            for b in range(N_CORES)
        ],
        axis=0,
    )
    ret = np.concatenate(
        [r.results[b]["ret"].reshape(-1) for b in range(N_CORES)], axis=0
    ).astype(np.int32)
    return scores, ret, r


def kernel(**inputs):
    scores, ret, _ = _run(inputs, trace=False)
    return scores, ret


if __name__ == "__main__":
    nc = _get_nc()
    print("built + compiled OK")


# revision 41
# speedup vs baseline: 2.2960x; 1.0921x over previous
"""Arcface pairwise-similarity loss kernel for one TRN2 chip (8 NeuronCores).

Reference computation (per batch element b):
  xn = x / ||x||_2 (over H)                      x: [S=5, W=1024, H=128]
  for each pair p=(i,j) of the 5 S-slices:
    scores[p] = 5 * xn[i] @ xn[j].T              -> [W, W]
    ret[p][w] = first v with target[j][v] == target[i][w], else 0

Sharding: data-parallel over B=8, one batch element per NeuronCore. No
cross-core communication; outputs are concatenated on the host.

Per-core device pipeline:
  Stage A: one DMA per s loads x[s] as [128, 8, 128] (partition = w%128);
           sum of squares / sqrt / reciprocal / scale are batched over
           the whole s (broadcast access patterns), then 8 PE transposes
           build xt[s] = [128(H), 1024(W)] bf16 matmul operands.
  Stage B: 10 pairs x 8 m-tiles x 2 bf16 matmuls [K=128, M=128, N=512].
           Full [128, 1024] PSUM tiles are evacuated with the x5 scale
           folded in, alternating ScalarE / VectorE; 4 m-tiles are
           staged per SBUF buffer so each output DMA writes a
           contiguous 2 MiB block.
  Stage C: ret_target without the O(W^2) argmax: per s a one-hot class
           map OH[c, v] = (target[s][v] == c) (64 classes, fp16). The
           first matching index per class depends only on j: 4 masked
           min-reductions. The per-w gather is an fp16 matmul grouped
           by i (lhsT = ADJS[:, i:4]), exact because integers <= 2048
           are exact in fp16.
"""

import os
import sys

if "/opt/trn_rl_repo" not in sys.path:
    sys.path.insert(0, "/opt/trn_rl_repo")

import numpy as np

B, S, W, H = 8, 5, 1024, 128
PAIRS = [(0, 1), (0, 2), (0, 3), (0, 4), (1, 2), (1, 3), (1, 4), (2, 3), (2, 4), (3, 4)]
P = len(PAIRS)
N_CORES = 8
NCLASS = 64
BIG = 2048.0  # > any valid index; exact in fp16/fp32

MM_DT = os.environ.get("K_MM_DT", "bf16")  # "bf16" | "f32r"

_CACHE = {}


def _build():
    import concourse.bass as bass
    import concourse.tile as tile
    from concourse import bacc, mybir
    from concourse.masks import make_identity

    f32 = mybir.dt.float32
    f16 = mybir.dt.float16
    mm_dt = mybir.dt.bfloat16 if MM_DT == "bf16" else mybir.dt.float32r
    i32 = mybir.dt.int32
    Alu = mybir.AluOpType
    Act = mybir.ActivationFunctionType

    nc = bacc.Bacc("TRN2", target_bir_lowering=False, debug=False, num_devices=N_CORES)

    x_h = nc.dram_tensor("x", [S, W, H], f32, kind="ExternalInput")
    t_h = nc.dram_tensor("tgt", [S, W], f32, kind="ExternalInput")
    out_dt = f32 if os.environ.get("K_OUT_DT", "bf16") == "f32" else mybir.dt.bfloat16
    sc_h = nc.dram_tensor("scores", [P, W, W], out_dt, kind="ExternalOutput")
    rt_h = nc.dram_tensor("ret", [P, W], i32, kind="ExternalOutput")

    MT = W // 128  # 8 m-tiles per pair
    MG = 4  # m-tiles staged per output DMA

    with tile.TileContext(nc) as tc:
        with (
            tc.tile_pool(name="persist", bufs=1) as persist,
            tc.tile_pool(name="xin", bufs=2) as xin_pool,
            tc.tile_pool(name="scr", bufs=3) as scr_pool,
            tc.tile_pool(name="small", bufs=4) as small_pool,
            tc.tile_pool(name="outp", bufs=10) as out_pool,
            tc.tile_pool(name="ps", bufs=4, space="PSUM") as ps_pool,
            tc.tile_pool(name="tb", bufs=2) as tb_pool,
        ):
            ident = persist.tile([128, 128], mm_dt, tag="ident")
            make_identity(nc, ident)

            # HAM warm-up: ~12us of throwaway matmuls issued while stage A
            # loads/normalizes, so the PE clock gate is at 8/8 when the
            # real matmul stream starts (transposes don't count as
            # PE-busy for HAM).
            wrhs = persist.tile([128, 512], mm_dt, tag="wrhs")
            nc.vector.memset(wrhs, 0.0)
            wps = ps_pool.tile([128, 512], f32, tag="ps")
            for _ in range(28):
                nc.tensor.matmul(wps, ident, wrhs, start=True, stop=True)

            # viota[c, v] = v - BIG (constant across partitions)
            viota = persist.tile([NCLASS, W], f16, tag="viota")
            nc.gpsimd.iota(
                viota,
                pattern=[[1, W]],
                base=int(-BIG),
                channel_multiplier=0,
                allow_small_or_imprecise_dtypes=True,
            )
            # ciota[c, 0] = c
            ciota = persist.tile([NCLASS, 1], f32, tag="ciota")
            nc.gpsimd.iota(
                ciota,
                pattern=[[1, 1]],
                base=0,
                channel_multiplier=1,
                allow_small_or_imprecise_dtypes=True,
            )

            xt = [
                persist.tile([128, W], mm_dt, tag=f"xt{s}", name=f"xt{s}")
                for s in range(S)
            ]
            oh = [
                persist.tile([NCLASS, W], f16, tag=f"oh{s}", name=f"oh{s}")
                for s in range(S)
            ]

            # ---- Stage A + B interleaved: as soon as both slices of a
            # pair are normalized+transposed, emit its matmuls so output
            # DMAs start early. ----
            def stage_a(s):
                xin = xin_pool.tile([128, MT, H], f32, tag="xin")
                # partition = w % 128, free = (m, h)
                nc.sync.dma_start(
                    out=xin, in_=x_h[s].rearrange("(m q) h -> q m h", q=128)
                )
                scr = scr_pool.tile([128, MT, H], f32, tag="scr")
                nc.vector.tensor_mul(scr, xin, xin)
                ss = small_pool.tile([128, MT], f32, tag="ss")
                nc.vector.tensor_reduce(ss, scr, axis=mybir.AxisListType.X, op=Alu.add)
                sq = small_pool.tile([128, MT], f32, tag="sq")
                nc.scalar.activation(out=sq, in_=ss, func=Act.Sqrt)
                rn = small_pool.tile([128, MT], f32, tag="rn")
                nc.vector.reciprocal(out=rn, in_=sq)
                # xn = x * rn, rn broadcast along h
                rnb = bass.AP(
                    tensor=rn.tensor,
                    offset=rn.offset,
                    ap=[*rn.ap, [0, H]],
                )
                xn = scr_pool.tile([128, MT, H], mm_dt, tag="xn")
                nc.vector.tensor_mul(xn, xin, rnb)
                for m in range(MT):
                    pt = ps_pool.tile([128, 128], mm_dt, tag="ps")
                    nc.tensor.transpose(pt, xn[:, m, :], ident)
                    nc.any.tensor_copy(out=xt[s][:, m * 128 : (m + 1) * 128], in_=pt)
                # one-hot class map for this s (fp16)
                tb = tb_pool.tile([NCLASS, W], f32, tag="tb")
                nc.gpsimd.dma_start(
                    out=tb, in_=t_h[s : s + 1, :].to_broadcast([NCLASS, W])
                )
                nc.vector.tensor_scalar(
                    out=oh[s], in0=tb, scalar1=ciota, scalar2=None, op0=Alu.is_equal
                )

            ndma = 0

            def mm_tile(p, i, j, m):
                # one [128, 1024] output tile of pair p at row block m
                nonlocal ndma
                msl = slice(m * 128, (m + 1) * 128)
                ps = ps_pool.tile([128, W], f32, tag="ps")
                lhsT = xt[i][:, msl]
                nc.tensor.matmul(
                    ps[:, 0:512], lhsT, xt[j][:, 0:512], start=True, stop=True
                )
                nc.tensor.matmul(
                    ps[:, 512:1024], lhsT, xt[j][:, 512:1024], start=True, stop=True
                )
                so = out_pool.tile([128, W], out_dt, tag="so")
                if ndma % 2 == 0:
                    nc.scalar.activation(out=so, in_=ps, func=Act.Copy, scale=5.0)
                else:
                    nc.vector.tensor_scalar_mul(so, ps, 5.0)
                eng = (nc.sync, nc.scalar, nc.gpsimd)[ndma % 3]
                ndma += 1
                eng.dma_start(out=sc_h[p, msl, :], in_=so)

            pair_idx = {pr: n for n, pr in enumerate(PAIRS)}
            for s in range(S):
                stage_a(s)
                for i in range(s):
                    for m in range(MT):
                        mm_tile(pair_idx[(i, s)], i, s, m)

            # ---- Stage C part 2: first-match indices (per s), gather by i ----
            FIM = persist.tile([NCLASS, 4], f16, tag="FIM")
            for s in range(1, 5):
                scv = scr_pool.tile([NCLASS, W], f16, tag="scv")
                # scv[c,v] = OH[s][c,v] * (v - BIG); min_v -> first idx - BIG
                nc.vector.tensor_mul(scv, oh[s], viota)
                nc.vector.tensor_reduce(
                    FIM[:, s - 1 : s], scv, axis=mybir.AxisListType.X, op=Alu.min
                )
            FI = small_pool.tile([NCLASS, 4], f16, tag="FI")
            nc.vector.tensor_scalar_add(FI, FIM, BIG)  # first idx, or BIG
            IND = small_pool.tile([NCLASS, 4], f16, tag="IND")
            nc.vector.tensor_scalar(
                out=IND, in0=FI, scalar1=BIG, scalar2=None, op0=Alu.is_lt
            )
            ADJS = small_pool.tile([NCLASS, 4], f16, tag="ADJS")
            nc.vector.tensor_mul(ADJS, FI, IND)  # BIG -> 0 (argmax default)

            row_off = [0, 4, 7, 9]
            for i in range(4):
                rows = 4 - i
                psg = ps_pool.tile([rows, W], f32, tag="ps")
                # out rows r -> pair (i, i+1+r); PAIRS order preserved
                nc.tensor.matmul(
                    psg[:, 0:512], ADJS[:, i:4], oh[i][:, 0:512],
                    start=True, stop=True,
                )
                nc.tensor.matmul(
                    psg[:, 512:1024], ADJS[:, i:4], oh[i][:, 512:1024],
                    start=True, stop=True,
                )
                ri = small_pool.tile([rows, W], i32, tag=f"ri{i}", name=f"ri{i}")
                nc.vector.tensor_copy(out=ri, in_=psg)
                nc.sync.dma_start(
                    out=rt_h[row_off[i] : row_off[i] + rows, :], in_=ri
                )

    nc.compile()
    return nc


def _get_nc():
    if "nc" not in _CACHE:
        _CACHE["nc"] = _build()
    return _CACHE["nc"]


def _run(inputs, trace=False, **kw):
    from concourse.bass_utils import run_bass_kernel_spmd

    nc = _get_nc()
    x = np.ascontiguousarray(np.asarray(inputs["x"], dtype=np.float32))
    t = np.ascontiguousarray(np.asarray(inputs["target"]).astype(np.float32))
    assert x.shape == (B, S, W, H) and t.shape == (B, S, W)
    in_maps = [{"x": x[b], "tgt": t[b]} for b in range(N_CORES)]
    r = run_bass_kernel_spmd(nc, in_maps, list(range(N_CORES)), trace=trace, **kw)
    scores = np.concatenate(
        [
            np.asarray(r.results[b]["scores"], dtype=np.float32).reshape(P * W, W)
            for b in range(N_CORES)
        ],
        axis=0,
    )
    ret = np.concatenate(
        [r.results[b]["ret"].reshape(-1) for b in range(N_CORES)], axis=0
    ).astype(np.int32)
    return scores, ret, r


def kernel(**inputs):
    scores, ret, _ = _run(inputs, trace=False)
    return scores, ret


if __name__ == "__main__":
    nc = _get_nc()
    print("built + compiled OK")
